# revision 31
# baseline (speedup 1.0000x reference)
"""Causal multi-head attention on 8 Trainium2 NeuronCores.

Sharding: data-parallel over batch (B=2) x tensor-parallel over heads
(16 heads -> 4 groups of 4). Core c handles batch c//4, head group c%4.
Each core computes q/k/v projections for its 4 heads, causal flash
attention, and a partial output projection (row slice of Wo); the host
sums the 4 partials per batch element.

All transposes happen on the HOST: the kernel receives x^T, wq^T, wk^T,
wv^T, wo^T pre-laid-out so every matmul operand DMAs straight into its
streaming layout. This removes ~190 PE transpose-mode instructions and
their psum->sbuf copies from the device critical path.

Matmuls run in bf16 (fp32 PSUM accumulation). QK^T scores are written
to PSUM in bf16 (softmax tolerates the rounding; halves score-bank
pressure). The softmax row-sum is fused into the o^T = [v|1s]^T P^T
matmul via an appended ones column; normalization (broadcast rowsum via
K=1 matmul reading partition 64, fast-approx reciprocal, divide) stays
in fp32. The y^T partials leave the device in bf16.

Phase 2 is software-pipelined at instruction-emission level: per
(q-chunk, head-pair) only the S^T = k q^T matmuls and the masked exp
are emitted in the main loop; AV matmuls, normalization epilogues, and
output projections drain from a work queue between them. AV units are
queued as soon as their exp is emitted (with a 3-slot lag guard) so the
final window self-drains instead of leaving a serial tail.
"""

import numpy as np
import ml_dtypes

import concourse.bacc as bacc
import concourse.bass as bass
import concourse.tile as tile
from concourse import bass_utils, mybir

B, S, D, H = 2, 2048, 1024, 16
DK = 64
NH = 4                 # heads per core
E = NH * DK            # 256: per-core head-dim slice
SCALE = 1.0 / 8.0      # 1/sqrt(DK)

F32 = mybir.dt.float32
F32R = mybir.dt.float32r
BF16 = mybir.dt.bfloat16

QC = 512               # q-chunk (columns per attention tile)
NQC = S // QC          # 4
NKB = S // 128         # 16 k-blocks


def _emit(tc, nc, xT_d, wqT_d, wkT_d, wvT_d, woT_d, yT_d, mask_d, ones_d):
    const = tc.alloc_tile_pool(name="const", bufs=1)
    perm = tc.alloc_tile_pool(name="perm", bufs=1)
    p01 = tc.alloc_tile_pool(name="p01", bufs=1)

    mask = const.tile([128, 128], BF16)
    ones_f32 = const.tile([128, 64], F32)
    ones128 = const.tile([128, 64], F32R)

    woT = perm.tile([128, 2, D], BF16)   # woT[p, ec, o] = wo[o, ec*128+p]
    qT = perm.tile([128, 2, S], BF16)    # qT[p, ec, s] = q[s, ec*128+p]
    kT = perm.tile([128, 2, S], BF16)
    v_sb = perm.tile([128, NKB, NH, DK + 1], BF16)  # [.., 64] = ones column

    xT = p01.tile([128, 8, S], BF16)     # xT[p, dc, s] = x[s, dc*128+p]
    wqT = p01.tile([128, 8, E], BF16)    # wqT[p, dc, e] = wq[e, dc*128+p]
    wkT = p01.tile([128, 8, E], BF16)
    wvT = p01.tile([128, 8, E], BF16)

    # startup DMAs batched into 4-dc groups (one descriptor each) and
    # spread across four engine queues so issue costs don't serialize:
    # the first projection chain needs wq + xT[sc=0] only
    def grp(dram, dcw, dc0, n, sc0=0, w=None):
        w = w if w is not None else dram.ap[0][0]
        return bass.AP(
            tensor=dram.tensor, offset=dc0 * 128 * w + sc0,
            ap=[[w, 128], [128 * w, n], [1, dcw]],
        )

    # wq split across two queues so its full dc chain lands first; wk next
    for dc in range(4):
        nc.scalar.dma_start(out=wqT[:, dc, :], in_=wqT_d[dc * 128:(dc + 1) * 128, :])
        nc.gpsimd.dma_start(out=wqT[:, dc + 4, :], in_=wqT_d[(dc + 4) * 128:(dc + 5) * 128, :])
    for dc in range(4):
        nc.scalar.dma_start(out=wkT[:, dc, :], in_=wkT_d[dc * 128:(dc + 1) * 128, :])
        nc.gpsimd.dma_start(out=wkT[:, dc + 4, :], in_=wkT_d[(dc + 4) * 128:(dc + 5) * 128, :])
    # sc=0 per-dc so the first projection chain starts ASAP; later chunks
    # as grouped transfers (one issue per 4 dc) to keep the sync queue free
    for dc in range(8):
        nc.sync.dma_start(
            out=xT[:, dc, 0:512],
            in_=xT_d[dc * 128:(dc + 1) * 128, 0:512],
        )
    for sc in range(1, 4):
        for dc0 in (0, 4):
            nc.sync.dma_start(
                out=xT[:, dc0:dc0 + 4, sc * 512:(sc + 1) * 512],
                in_=grp(xT_d, 512, dc0, 4, sc0=sc * 512),
            )
    nc.gpsimd.dma_start(out=wvT, in_=grp(wvT_d, E, 0, 8))
    nc.gpsimd.dma_start(out=woT, in_=grp(woT_d, D, 0, 2))
    nc.gpsimd.dma_start(out=mask, in_=mask_d)
    nc.gpsimd.dma_start(out=ones_f32, in_=ones_d)
    # ones row for the rowsum broadcast (row 64 used as lhsT)
    nc.vector.tensor_copy(ones128, ones_f32)

    ncopy = [0]

    def copy(dst, src):
        # alternate psum->sbuf copies between DVE and ACT
        if ncopy[0] % 2 == 0:
            nc.vector.tensor_copy(dst, src)
        else:
            nc.scalar.copy(dst, src)
        ncopy[0] += 1

    work = tc.alloc_tile_pool(name="work", bufs=3)
    small = tc.alloc_tile_pool(name="small", bufs=2)

    # ---- phases 1-3 fused: the attention windows are exp(ACT)-paced, so
    # the q/k/v projections (pure PE work) drain INTO the windows as
    # background fuel; window qc only needs proj chunks sc <= qc ----
    with tc.tile_pool(name="ps01", bufs=1, space="PSUM") as ps01, \
         tc.tile_pool(name="psS", bufs=1, space="PSUM") as ps_S, \
         tc.tile_pool(name="psO", bufs=1, space="PSUM") as ps_o:
        ps_y = ps01

        # ones column of v (written once; strided 3D AP)
        ones_ap = bass.AP(
            tensor=v_sb.tensor,
            offset=v_sb.offset + DK,
            ap=[v_sb.ap[0], [NH * (DK + 1), NKB], [DK + 1, NH]],
        )
        src64 = bass.AP(
            tensor=ones_f32.tensor, offset=ones_f32.offset,
            ap=[ones_f32.ap[0], [4, NKB], [1, NH]],
        )
        nc.vector.tensor_copy(ones_ap, src64)

        # touch exp once so the ~2.7us ACT table load happens during the
        # projection warm-up instead of stalling the first QK window
        warm = const.tile([1, 4], F32)
        nc.scalar.activation(
            warm, ones_f32[0:1, 0:4], mybir.ActivationFunctionType.Exp
        )

        def make_proj(w_t, outT, ec, sc):
            def u():
                ps = ps01.tile([128, 512], F32, tag="y", bufs=2, name="psp")
                for dc in range(8):
                    nc.tensor.matmul(
                        ps,
                        lhsT=w_t[:, dc, ec * 128:(ec + 1) * 128],
                        rhs=xT[:, dc, sc * 512:(sc + 1) * 512],
                        start=(dc == 0),
                        stop=(dc == 7),
                    )
                copy(outT[:, ec, sc * 512:(sc + 1) * 512], ps)
            return u

        projq = []
        for sc in range(4):
            for w_t, outT in [(wqT, qT), (wkT, kT)]:
                for ec in range(2):
                    if sc == 0:
                        make_proj(w_t, outT, ec, sc)()
                    else:
                        projq.append((sc, make_proj(w_t, outT, ec, sc)))

        # (weight, min_slot, closure): a unit may only drain once
        # slot_i >= min_slot (keeps AV >= 3 QK slots behind its exp, and
        # defers vproj(kb) to the first window that consumes it)
        workq = []
        slot_i = [0]
        drained = [0]

        def make_vproj(sblk):
            def u():
                ps = ps01.tile([128, E], F32, tag="y", bufs=2, name="psv")
                for dc in range(8):
                    nc.tensor.matmul(
                        ps,
                        lhsT=xT[:, dc, sblk * 128:(sblk + 1) * 128],
                        rhs=wvT[:, dc, :],
                        start=(dc == 0),
                        stop=(dc == 7),
                    )
                # scatter 4 heads into [.., l, 0:64]
                sap = bass.AP(
                    tensor=ps.tensor, offset=ps.offset,
                    ap=[ps.ap[0], [DK, NH], [1, DK]],
                )
                nc.vector.tensor_copy(v_sb[:, sblk, :, 0:DK], sap)
            return u

        # attention windows (q-column ranges): the last 512 columns run as
        # two 256-wide windows so the end-of-kernel epilogue + output
        # projection tail is half as long
        WINS = [(0, 512), (512, 512), (1024, 512), (1536, 256), (1792, 256)]
        _starts = []
        _acc = 0
        for (_q0, _w) in WINS:
            _starts.append(_acc)
            _acc += 2 * ((_q0 + _w) // 128)
        TOTAL_SLOTS = _acc                                           # 108

        def _vp_start(kb):
            for (s, (_q0, _w)) in zip(_starts, WINS):
                if kb * 128 < _q0 + _w:
                    return s
            return 0

        for sblk in range(NKB):
            workq.append((2, _vp_start(sblk), make_vproj(sblk)))

        # drain pacing: spread the total background work evenly over all
        # QK slots; scan past not-yet-eligible units (safe: relative order
        # of dependent units is preserved by min_slot construction)
        W_TOTAL = (4 * len(projq) + 2 * NKB + TOTAL_SLOTS + 2 * len(WINS)
                   + 8 * len(WINS))

        def drain_some(budget_cap=5):
            target = (W_TOTAL * slot_i[0]) // TOTAL_SLOTS
            # hold back a fuel reserve so the final windows' QK slots still
            # have matmul work to interleave (prevents the late HAM cool)
            if slot_i[0] < _starts[-1]:
                target -= 12
            budget = min(budget_cap, target - drained[0])
            j = 0
            while j < len(workq) and budget > 0:
                if workq[j][1] <= slot_i[0]:
                    w, _, u = workq.pop(j)
                    u()
                    drained[0] += w
                    budget -= w
                else:
                    j += 1
            if budget > 0 and projq:
                _, u = projq.pop(0)
                u()
                drained[0] += 4

        def make_av(po_box, pts, kb, hp, kmax, width):
            last = kb == kmax - 1

            def av():
                if po_box[0] is None:
                    po_box[0] = (
                        ps_o.tile([DK + 1, QC], F32, tag="o", bufs=2, name="poA"),
                        ps_o.tile([DK + 1, QC], F32, tag="o", bufs=2, name="poB"),
                    )
                poA, poB = po_box[0]
                pT, cs = pts[kb]
                for hi, po in ((0, poA), (1, poB)):
                    nc.tensor.matmul(
                        po[:, cs:width],
                        lhsT=v_sb[:, kb, 2 * hp + hi, :],
                        rhs=pT[:, hi, cs:width],
                        start=(kb == 0),
                        stop=last,
                    )
                if last:
                    # free the psum banks + stage the rowsum rows for the
                    # broadcast matmul; emitting here hides the copy
                    # latency before the epilogue's bc matmul runs. One
                    # copy per engine so they run in parallel. Head B's
                    # o^T is also shifted to partitions 64-127 now, off
                    # the epilogue critical path.
                    oA_sb = small.tile([DK + 1, QC], F32R, tag="osb", bufs=4)
                    oB_sb = small.tile([DK + 1, QC], F32R, tag="osb", bufs=4)
                    nc.scalar.copy(oA_sb[:, 0:width], poA[:, 0:width])
                    nc.vector.tensor_copy(oB_sb[:, 0:width], poB[:, 0:width])
                    po_box[1] = (oA_sb, oB_sb)
            return av

        def make_epilogue(po_box, oT, hp, width):
            def epi():
                oA_sb, oB_sb = po_box[1]
                bcs = []
                for o_sb in (oA_sb, oB_sb):
                    ps_bc = ps_y.tile([64, QC], F32, tag="y", bufs=2, name="psbc")
                    nc.tensor.matmul(
                        ps_bc[:, 0:width],
                        lhsT=ones128[64:65, :],
                        rhs=o_sb[DK:DK + 1, 0:width],
                        start=True,
                        stop=True,
                    )
                    bcs.append(ps_bc)
                recs = []
                for ps_bc in bcs:
                    rec = small.tile([64, QC], F32, tag="rec", bufs=2)
                    nc.vector.reciprocal_approx_fast(rec[:, 0:width], ps_bc[:, 0:width])
                    recs.append(rec)
                nc.vector.tensor_mul(
                    oT[0:64, hp, 0:width], oA_sb[0:DK, 0:width], recs[0][:, 0:width]
                )
                tmpB = small.tile([64, QC], BF16, tag="tmpB", bufs=2)
                nc.vector.tensor_mul(
                    tmpB[:, 0:width], oB_sb[0:DK, 0:width], recs[1][:, 0:width]
                )
                nc.gpsimd.dma_start(
                    out=oT[64:128, hp, 0:width], in_=tmpB[:, 0:width]
                )
            return epi

        def make_out_proj(q0, width, is_last, oT):
            units = []
            for dc in range(8):
                def u(dc=dc, oT=oT):
                    if is_last:
                        # final window: QK is done, so the score banks are
                        # free — run the tail's projections there to
                        # decouple from the epilogue's bc/recip traffic
                        psyt = ps_S.tile([128, 2, 512], F32, tag="S", bufs=2,
                                         name="psyt")
                        psy = psyt[:, 0, 0:width]
                    else:
                        psyf = ps_y.tile([128, QC], F32, tag="y", bufs=2,
                                         name="psy")
                        psy = psyf[:, 0:width]
                    for ec in range(2):
                        nc.tensor.matmul(
                            psy,
                            lhsT=woT[:, ec, dc * 128:(dc + 1) * 128],
                            rhs=oT[:, ec, 0:width],
                            start=(ec == 0),
                            stop=(ec == 1),
                        )
                    y_sb = work.tile([128, QC], BF16, tag="ysb", bufs=3)
                    # strict dc-parity alternation: consecutive units'
                    # copies land on different engines and overlap
                    if dc % 2 == 0:
                        nc.vector.tensor_copy(y_sb[:, 0:width], psy)
                    else:
                        nc.scalar.copy(y_sb[:, 0:width], psy)
                    nc.sync.dma_start(
                        out=yT_d[dc * 128:(dc + 1) * 128, q0:q0 + width],
                        in_=y_sb[:, 0:width],
                    )
                units.append(u)
            return units

        for wi, (q0, width) in enumerate(WINS):
            # this window reads qT/kT chunks covering columns < q0+width:
            # force any not yet drained (usually already gone as fuel)
            sc_need = (q0 + width - 1) // 512
            while projq and projq[0][0] <= sc_need:
                projq.pop(0)[1]()
            oT = work.tile([128, 2, QC], BF16, tag="oT", bufs=2)
            kmax = (q0 + width) // 128
            is_last = wi == len(WINS) - 1
            for hp in range(2):
                pts = {}
                po_box = [None, None]
                for kb in range(kmax):
                    # S^T = k q^T, 2-head row-tiled pair, causally narrowed
                    cs = max(0, kb * 128 - q0)
                    psS = ps_S.tile([128, 2, 512], F32, tag="S", bufs=2)
                    for hi in range(2):
                        nc.tensor.matmul(
                            psS[:, hi, cs:width],
                            lhsT=kT[hi * 64:(hi + 1) * 64, hp,
                                    kb * 128:(kb + 1) * 128],
                            rhs=qT[hi * 64:(hi + 1) * 64, hp,
                                   q0 + cs:q0 + width],
                            start=True,
                            stop=True,
                        )
                    pT = work.tile([128, 2, 512], BF16, tag="pT", bufs=24)
                    pts[kb] = (pT, cs)
                    nc.scalar.activation(
                        pT[:, :, cs:width],
                        psS[:, :, cs:width],
                        mybir.ActivationFunctionType.Exp,
                        scale=SCALE,
                    )
                    if kb * 128 >= q0:  # diagonal band: zero upper triangle
                        # 0/1 multiply AFTER exp, on the deeply-buffered pT;
                        # one DVE op for both heads via a stride-0 middle dim
                        mask2 = bass.AP(
                            tensor=mask.tensor, offset=mask.offset,
                            ap=[mask.ap[0], [0, 2], mask.ap[1]],
                        )
                        nc.vector.tensor_mul(
                            pT[:, :, cs:cs + 128],
                            pT[:, :, cs:cs + 128],
                            mask2,
                        )
                    # AV for this kb becomes available 3 QK slots later
                    workq.append(
                        (1, slot_i[0] + 3, make_av(po_box, pts, kb, hp, kmax, width))
                    )
                    slot_i[0] += 1
                    drain_some()
                # epi min matches its last AV's eligibility; outproj one
                # later (list order breaks the tie in favor of epi)
                workq.append((1, slot_i[0] + 2, make_epilogue(po_box, oT, hp, width)))
                if hp == 1:
                    workq.extend(
                        (1, slot_i[0] + 3, u)
                        for u in make_out_proj(q0, width, is_last, oT)
                    )
        for _, _, u in workq:
            u()

    for p in [small, work, p01, perm, const]:
        p.release()


_CACHE = {}


def _build():
    if "nc" in _CACHE:
        return _CACHE["nc"]
    nc = bacc.Bacc("TRN2", target_bir_lowering=False, debug=False, num_devices=8)
    xT_d = nc.dram_tensor("xT", [D, S], BF16, kind="ExternalInput").ap()
    wqT_d = nc.dram_tensor("wqT", [D, E], BF16, kind="ExternalInput").ap()
    wkT_d = nc.dram_tensor("wkT", [D, E], BF16, kind="ExternalInput").ap()
    wvT_d = nc.dram_tensor("wvT", [D, E], BF16, kind="ExternalInput").ap()
    woT_d = nc.dram_tensor("woT", [E, D], BF16, kind="ExternalInput").ap()
    yT_d = nc.dram_tensor("yT", [D, S], BF16, kind="ExternalOutput").ap()
    mask_d = nc.dram_tensor("maskc", [128, 128], BF16, kind="ExternalInput").ap()
    ones_d = nc.dram_tensor("onesc", [128, 64], F32, kind="ExternalInput").ap()
    with tile.TileContext(nc) as tc:
        _emit(tc, nc, xT_d, wqT_d, wkT_d, wvT_d, woT_d, yT_d, mask_d, ones_d)
    nc.compile()
    _CACHE["nc"] = nc
    return nc


_r = np.arange(128)
_MASK = np.where(_r[:, None] <= _r[None, :], 1.0, 0.0).astype(ml_dtypes.bfloat16)
_ONES = np.ones((128, 64), dtype=np.float32)

LAST_RESULT = None


def kernel(x, wq, wk, wv, wo):
    global LAST_RESULT
    nc = _build()
    bf = ml_dtypes.bfloat16
    x = np.asarray(x, dtype=np.float32)
    wq = np.asarray(wq, dtype=np.float32)
    wk = np.asarray(wk, dtype=np.float32)
    wv = np.asarray(wv, dtype=np.float32)
    wo = np.asarray(wo, dtype=np.float32)

    in_maps = []
    for c in range(8):
        b, g = c // 4, c % 4
        rows = slice(g * E, (g + 1) * E)
        in_maps.append({
            "xT": np.ascontiguousarray(x[b].T.astype(bf)),
            "wqT": np.ascontiguousarray(wq[rows].T.astype(bf)),
            "wkT": np.ascontiguousarray(wk[rows].T.astype(bf)),
            "wvT": np.ascontiguousarray(wv[rows].T.astype(bf)),
            "woT": np.ascontiguousarray(wo[:, rows].T.astype(bf)),
            "maskc": _MASK,
            "onesc": _ONES,
        })

    res = bass_utils.run_bass_kernel_spmd(nc, in_maps, core_ids=list(range(8)))
    LAST_RESULT = res

    y = np.empty((B, S, D), dtype=np.float32)
    for b in range(B):
        acc = res.results[4 * b]["yT"].astype(np.float32)
        for g in range(1, 4):
            acc += res.results[4 * b + g]["yT"].astype(np.float32)
        y[b] = acc.T
    return y


# revision 32
# speedup vs baseline: 1.0444x; 1.0444x over previous
"""Causal multi-head attention on 8 Trainium2 NeuronCores.

Sharding: data-parallel over batch (B=2) x tensor-parallel over heads
(16 heads -> 4 groups of 4). Core c handles batch c//4, head group c%4.
Each core computes q/k/v projections for its 4 heads, causal flash
attention, and a partial output projection (row slice of Wo); the host
sums the 4 partials per batch element.

All transposes happen on the HOST: the kernel receives x^T, wq^T, wk^T,
wv^T, wo^T pre-laid-out so every matmul operand DMAs straight into its
streaming layout. This removes ~190 PE transpose-mode instructions and
their psum->sbuf copies from the device critical path.

Matmuls run in bf16 (fp32 PSUM accumulation). QK^T scores are written
to PSUM in bf16 (softmax tolerates the rounding; halves score-bank
pressure). The softmax row-sum is fused into the o^T = [v|1s]^T P^T
matmul via an appended ones column; normalization (broadcast rowsum via
K=1 matmul reading partition 64, fast-approx reciprocal, divide) stays
in fp32. The y^T partials leave the device in bf16.

Phase 2 is software-pipelined at instruction-emission level: per
(q-chunk, head-pair) only the S^T = k q^T matmuls and the masked exp
are emitted in the main loop; AV matmuls, normalization epilogues, and
output projections drain from a work queue between them. AV units are
queued as soon as their exp is emitted (with a 3-slot lag guard) so the
final window self-drains instead of leaving a serial tail.
"""

import numpy as np
import ml_dtypes

import concourse.bacc as bacc
import concourse.bass as bass
import concourse.tile as tile
from concourse import bass_utils, mybir

B, S, D, H = 2, 2048, 1024, 16
DK = 64
NH = 4                 # heads per core
E = NH * DK            # 256: per-core head-dim slice
SCALE = 1.0 / 8.0      # 1/sqrt(DK)

F32 = mybir.dt.float32
F32R = mybir.dt.float32r
BF16 = mybir.dt.bfloat16

QC = 512               # q-chunk (columns per attention tile)
NQC = S // QC          # 4
NKB = S // 128         # 16 k-blocks


def _emit(tc, nc, xT_d, wqT_d, wkT_d, wvT_d, woT_d, yT_d, mask_d, ones_d):
    const = tc.alloc_tile_pool(name="const", bufs=1)
    perm = tc.alloc_tile_pool(name="perm", bufs=1)
    p01 = tc.alloc_tile_pool(name="p01", bufs=1)

    mask = const.tile([128, 128], BF16)
    ones_f32 = const.tile([128, 64], F32)
    ones128 = const.tile([128, 64], F32R)

    woT = perm.tile([128, 2, D], BF16)   # woT[p, ec, o] = wo[o, ec*128+p]
    qT = perm.tile([128, 2, S], BF16)    # qT[p, ec, s] = q[s, ec*128+p]
    kT = perm.tile([128, 2, S], BF16)
    v_sb = perm.tile([128, NKB, NH, DK + 1], BF16)  # [.., 64] = ones column

    xT = p01.tile([128, 8, S], BF16)     # xT[p, dc, s] = x[s, dc*128+p]
    wqT = p01.tile([128, 8, E], BF16)    # wqT[p, dc, e] = wq[e, dc*128+p]
    wkT = p01.tile([128, 8, E], BF16)
    wvT = p01.tile([128, 8, E], BF16)

    # startup DMAs batched into 4-dc groups (one descriptor each) and
    # spread across four engine queues so issue costs don't serialize:
    # the first projection chain needs wq + xT[sc=0] only
    def grp(dram, dcw, dc0, n, sc0=0, w=None):
        w = w if w is not None else dram.ap[0][0]
        return bass.AP(
            tensor=dram.tensor, offset=dc0 * 128 * w + sc0,
            ap=[[w, 128], [128 * w, n], [1, dcw]],
        )

    # wq split across two queues so its full dc chain lands first; wk next
    for dc in range(4):
        nc.scalar.dma_start(out=wqT[:, dc, :], in_=wqT_d[dc * 128:(dc + 1) * 128, :])
        nc.gpsimd.dma_start(out=wqT[:, dc + 4, :], in_=wqT_d[(dc + 4) * 128:(dc + 5) * 128, :])
    for dc in range(4):
        nc.scalar.dma_start(out=wkT[:, dc, :], in_=wkT_d[dc * 128:(dc + 1) * 128, :])
        nc.gpsimd.dma_start(out=wkT[:, dc + 4, :], in_=wkT_d[(dc + 4) * 128:(dc + 5) * 128, :])
    for sc in range(4):
        for dc in range(8):
            nc.sync.dma_start(
                out=xT[:, dc, sc * 512:(sc + 1) * 512],
                in_=xT_d[dc * 128:(dc + 1) * 128, sc * 512:(sc + 1) * 512],
            )
    nc.gpsimd.dma_start(out=wvT, in_=grp(wvT_d, E, 0, 8))
    nc.gpsimd.dma_start(out=woT, in_=grp(woT_d, D, 0, 2))
    nc.gpsimd.dma_start(out=mask, in_=mask_d)
    nc.gpsimd.dma_start(out=ones_f32, in_=ones_d)
    # ones row for the rowsum broadcast (row 64 used as lhsT)
    nc.vector.tensor_copy(ones128, ones_f32)

    ncopy = [0]

    def copy(dst, src):
        # alternate psum->sbuf copies between DVE and ACT
        if ncopy[0] % 2 == 0:
            nc.vector.tensor_copy(dst, src)
        else:
            nc.scalar.copy(dst, src)
        ncopy[0] += 1

    work = tc.alloc_tile_pool(name="work", bufs=3)
    small = tc.alloc_tile_pool(name="small", bufs=2)

    # ---- phases 1-3 fused: the attention windows are exp(ACT)-paced, so
    # the q/k/v projections (pure PE work) drain INTO the windows as
    # background fuel; window qc only needs proj chunks sc <= qc ----
    with tc.tile_pool(name="ps01", bufs=1, space="PSUM") as ps01, \
         tc.tile_pool(name="psS", bufs=1, space="PSUM") as ps_S, \
         tc.tile_pool(name="psO", bufs=1, space="PSUM") as ps_o:
        ps_y = ps01

        # ones column of v (written once; strided 3D AP)
        ones_ap = bass.AP(
            tensor=v_sb.tensor,
            offset=v_sb.offset + DK,
            ap=[v_sb.ap[0], [NH * (DK + 1), NKB], [DK + 1, NH]],
        )
        src64 = bass.AP(
            tensor=ones_f32.tensor, offset=ones_f32.offset,
            ap=[ones_f32.ap[0], [4, NKB], [1, NH]],
        )
        nc.vector.tensor_copy(ones_ap, src64)

        # touch exp once so the ~2.7us ACT table load happens during the
        # projection warm-up instead of stalling the first QK window
        warm = const.tile([1, 4], F32)
        nc.scalar.activation(
            warm, ones_f32[0:1, 0:4], mybir.ActivationFunctionType.Exp
        )

        def make_proj(w_t, outT, ec, sc):
            def u():
                ps = ps01.tile([128, 512], F32, tag="y", bufs=2, name="psp")
                for dc in range(8):
                    nc.tensor.matmul(
                        ps,
                        lhsT=w_t[:, dc, ec * 128:(ec + 1) * 128],
                        rhs=xT[:, dc, sc * 512:(sc + 1) * 512],
                        start=(dc == 0),
                        stop=(dc == 7),
                    )
                copy(outT[:, ec, sc * 512:(sc + 1) * 512], ps)
            return u

        projq = []
        for sc in range(4):
            for w_t, outT in [(wqT, qT), (wkT, kT)]:
                for ec in range(2):
                    if sc == 0:
                        make_proj(w_t, outT, ec, sc)()
                    else:
                        projq.append((sc, make_proj(w_t, outT, ec, sc)))

        # (weight, min_slot, closure): a unit may only drain once
        # slot_i >= min_slot (keeps AV >= 3 QK slots behind its exp, and
        # defers vproj(kb) to the first window that consumes it)
        workq = []
        slot_i = [0]
        drained = [0]

        def make_vproj(sblk):
            def u():
                ps = ps01.tile([128, E], F32, tag="y", bufs=2, name="psv")
                for dc in range(8):
                    nc.tensor.matmul(
                        ps,
                        lhsT=xT[:, dc, sblk * 128:(sblk + 1) * 128],
                        rhs=wvT[:, dc, :],
                        start=(dc == 0),
                        stop=(dc == 7),
                    )
                # scatter 4 heads into [.., l, 0:64]
                sap = bass.AP(
                    tensor=ps.tensor, offset=ps.offset,
                    ap=[ps.ap[0], [DK, NH], [1, DK]],
                )
                nc.vector.tensor_copy(v_sb[:, sblk, :, 0:DK], sap)
            return u

        # attention windows (q-column ranges)
        WINS = [(0, 512), (512, 512), (1024, 512), (1536, 512)]
        _starts = []
        _acc = 0
        for (_q0, _w) in WINS:
            _starts.append(_acc)
            _acc += 2 * ((_q0 + _w) // 128)
        TOTAL_SLOTS = _acc                                           # 108

        def _vp_start(kb):
            for (s, (_q0, _w)) in zip(_starts, WINS):
                if kb * 128 < _q0 + _w:
                    return s
            return 0

        for sblk in range(NKB):
            workq.append((2, _vp_start(sblk), make_vproj(sblk)))

        # drain pacing: spread the total background work evenly over all
        # QK slots; scan past not-yet-eligible units (safe: relative order
        # of dependent units is preserved by min_slot construction)
        W_TOTAL = (4 * len(projq) + 2 * NKB + TOTAL_SLOTS + 2 * len(WINS)
                   + 8 * len(WINS))

        def drain_some(budget_cap=5):
            target = (W_TOTAL * slot_i[0]) // TOTAL_SLOTS
            budget = min(budget_cap, target - drained[0])
            j = 0
            while j < len(workq) and budget > 0:
                if workq[j][1] <= slot_i[0]:
                    w, _, u = workq.pop(j)
                    u()
                    drained[0] += w
                    budget -= w
                else:
                    j += 1
            if budget > 0 and projq:
                _, u = projq.pop(0)
                u()
                drained[0] += 4

        def make_av(po_box, pts, kb, hp, kmax, width):
            last = kb == kmax - 1

            def av():
                if po_box[0] is None:
                    po_box[0] = (
                        ps_o.tile([DK + 1, QC], F32, tag="o", bufs=2, name="poA"),
                        ps_o.tile([DK + 1, QC], F32, tag="o", bufs=2, name="poB"),
                    )
                poA, poB = po_box[0]
                pT, cs = pts[kb]
                for hi, po in ((0, poA), (1, poB)):
                    nc.tensor.matmul(
                        po[:, cs:width],
                        lhsT=v_sb[:, kb, 2 * hp + hi, :],
                        rhs=pT[:, hi, cs:width],
                        start=(kb == 0),
                        stop=last,
                    )
                if last:
                    # free the psum banks + stage the rowsum rows for the
                    # broadcast matmul; emitting here hides the copy
                    # latency before the epilogue's bc matmul runs. One
                    # copy per engine so they run in parallel. Head B's
                    # o^T is also shifted to partitions 64-127 now, off
                    # the epilogue critical path.
                    oA_sb = small.tile([DK + 1, QC], F32R, tag="osb", bufs=4)
                    oB_sb = small.tile([DK + 1, QC], F32R, tag="osb", bufs=4)
                    nc.scalar.copy(oA_sb[:, 0:width], poA[:, 0:width])
                    nc.vector.tensor_copy(oB_sb[:, 0:width], poB[:, 0:width])
                    po_box[1] = (oA_sb, oB_sb)
            return av

        def make_epilogue(po_box, oT, hp, width):
            def epi():
                oA_sb, oB_sb = po_box[1]
                bcs = []
                for o_sb in (oA_sb, oB_sb):
                    ps_bc = ps_y.tile([64, QC], F32, tag="y", bufs=2, name="psbc")
                    nc.tensor.matmul(
                        ps_bc[:, 0:width],
                        lhsT=ones128[64:65, :],
                        rhs=o_sb[DK:DK + 1, 0:width],
                        start=True,
                        stop=True,
                    )
                    bcs.append(ps_bc)
                recs = []
                for ps_bc in bcs:
                    rec = small.tile([64, QC], F32, tag="rec", bufs=2)
                    nc.vector.reciprocal_approx_fast(rec[:, 0:width], ps_bc[:, 0:width])
                    recs.append(rec)
                nc.vector.tensor_mul(
                    oT[0:64, hp, 0:width], oA_sb[0:DK, 0:width], recs[0][:, 0:width]
                )
                tmpB = small.tile([64, QC], BF16, tag="tmpB", bufs=2)
                nc.vector.tensor_mul(
                    tmpB[:, 0:width], oB_sb[0:DK, 0:width], recs[1][:, 0:width]
                )
                nc.gpsimd.dma_start(
                    out=oT[64:128, hp, 0:width], in_=tmpB[:, 0:width]
                )
            return epi

        def make_out_proj(q0, width, is_last, oT):
            units = []
            for dc in range(8):
                def u(dc=dc, oT=oT):
                    if is_last:
                        # final window: QK is done, so the score banks are
                        # free — run the tail's projections there to
                        # decouple from the epilogue's bc/recip traffic
                        psyt = ps_S.tile([128, 2, 512], F32, tag="S", bufs=2,
                                         name="psyt")
                        psy = psyt[:, 0, 0:width]
                    else:
                        psyf = ps_y.tile([128, QC], F32, tag="y", bufs=2,
                                         name="psy")
                        psy = psyf[:, 0:width]
                    for ec in range(2):
                        nc.tensor.matmul(
                            psy,
                            lhsT=woT[:, ec, dc * 128:(dc + 1) * 128],
                            rhs=oT[:, ec, 0:width],
                            start=(ec == 0),
                            stop=(ec == 1),
                        )
                    y_sb = work.tile([128, QC], BF16, tag="ysb", bufs=3)
                    # strict dc-parity alternation: consecutive units'
                    # copies land on different engines and overlap
                    if dc % 2 == 0:
                        nc.vector.tensor_copy(y_sb[:, 0:width], psy)
                    else:
                        nc.scalar.copy(y_sb[:, 0:width], psy)
                    nc.sync.dma_start(
                        out=yT_d[dc * 128:(dc + 1) * 128, q0:q0 + width],
                        in_=y_sb[:, 0:width],
                    )
                units.append(u)
            return units

        for wi, (q0, width) in enumerate(WINS):
            # this window reads qT/kT chunks covering columns < q0+width:
            # force any not yet drained (usually already gone as fuel)
            sc_need = (q0 + width - 1) // 512
            while projq and projq[0][0] <= sc_need:
                projq.pop(0)[1]()
            oT = work.tile([128, 2, QC], BF16, tag="oT", bufs=2)
            kmax = (q0 + width) // 128
            is_last = wi == len(WINS) - 1
            for hp in range(2):
                pts = {}
                po_box = [None, None]
                for kb in range(kmax):
                    # S^T = k q^T, 2-head row-tiled pair, causally narrowed
                    cs = max(0, kb * 128 - q0)
                    psS = ps_S.tile([128, 2, 512], F32, tag="S", bufs=2)
                    for hi in range(2):
                        nc.tensor.matmul(
                            psS[:, hi, cs:width],
                            lhsT=kT[hi * 64:(hi + 1) * 64, hp,
                                    kb * 128:(kb + 1) * 128],
                            rhs=qT[hi * 64:(hi + 1) * 64, hp,
                                   q0 + cs:q0 + width],
                            start=True,
                            stop=True,
                        )
                    pT = work.tile([128, 2, 512], BF16, tag="pT", bufs=24)
                    pts[kb] = (pT, cs)
                    nc.scalar.activation(
                        pT[:, :, cs:width],
                        psS[:, :, cs:width],
                        mybir.ActivationFunctionType.Exp,
                        scale=SCALE,
                    )
                    if kb * 128 >= q0:  # diagonal band: zero upper triangle
                        # 0/1 multiply AFTER exp, on the deeply-buffered pT;
                        # one DVE op for both heads via a stride-0 middle dim
                        mask2 = bass.AP(
                            tensor=mask.tensor, offset=mask.offset,
                            ap=[mask.ap[0], [0, 2], mask.ap[1]],
                        )
                        nc.vector.tensor_mul(
                            pT[:, :, cs:cs + 128],
                            pT[:, :, cs:cs + 128],
                            mask2,
                        )
                    # AV for this kb becomes available 3 QK slots later
                    workq.append(
                        (1, slot_i[0] + 3, make_av(po_box, pts, kb, hp, kmax, width))
                    )
                    slot_i[0] += 1
                    drain_some()
                # epi min matches its last AV's eligibility; outproj one
                # later (list order breaks the tie in favor of epi)
                workq.append((1, slot_i[0] + 2, make_epilogue(po_box, oT, hp, width)))
                if hp == 1:
                    workq.extend(
                        (1, slot_i[0] + 3, u)
                        for u in make_out_proj(q0, width, is_last, oT)
                    )
        for _, _, u in workq:
            u()

    for p in [small, work, p01, perm, const]:
        p.release()


_CACHE = {}


def _build():
    if "nc" in _CACHE:
        return _CACHE["nc"]
    nc = bacc.Bacc("TRN2", target_bir_lowering=False, debug=False, num_devices=8)
    xT_d = nc.dram_tensor("xT", [D, S], BF16, kind="ExternalInput").ap()
    wqT_d = nc.dram_tensor("wqT", [D, E], BF16, kind="ExternalInput").ap()
    wkT_d = nc.dram_tensor("wkT", [D, E], BF16, kind="ExternalInput").ap()
    wvT_d = nc.dram_tensor("wvT", [D, E], BF16, kind="ExternalInput").ap()
    woT_d = nc.dram_tensor("woT", [E, D], BF16, kind="ExternalInput").ap()
    yT_d = nc.dram_tensor("yT", [D, S], BF16, kind="ExternalOutput").ap()
    mask_d = nc.dram_tensor("maskc", [128, 128], BF16, kind="ExternalInput").ap()
    ones_d = nc.dram_tensor("onesc", [128, 64], F32, kind="ExternalInput").ap()
    with tile.TileContext(nc) as tc:
        _emit(tc, nc, xT_d, wqT_d, wkT_d, wvT_d, woT_d, yT_d, mask_d, ones_d)
    nc.compile()
    _CACHE["nc"] = nc
    return nc


_r = np.arange(128)
_MASK = np.where(_r[:, None] <= _r[None, :], 1.0, 0.0).astype(ml_dtypes.bfloat16)
_ONES = np.ones((128, 64), dtype=np.float32)

LAST_RESULT = None


def kernel(x, wq, wk, wv, wo):
    global LAST_RESULT
    nc = _build()
    bf = ml_dtypes.bfloat16
    x = np.asarray(x, dtype=np.float32)
    wq = np.asarray(wq, dtype=np.float32)
    wk = np.asarray(wk, dtype=np.float32)
    wv = np.asarray(wv, dtype=np.float32)
    wo = np.asarray(wo, dtype=np.float32)

    in_maps = []
    for c in range(8):
        b, g = c // 4, c % 4
        rows = slice(g * E, (g + 1) * E)
        in_maps.append({
            "xT": np.ascontiguousarray(x[b].T.astype(bf)),
            "wqT": np.ascontiguousarray(wq[rows].T.astype(bf)),
            "wkT": np.ascontiguousarray(wk[rows].T.astype(bf)),
            "wvT": np.ascontiguousarray(wv[rows].T.astype(bf)),
            "woT": np.ascontiguousarray(wo[:, rows].T.astype(bf)),
            "maskc": _MASK,
            "onesc": _ONES,
        })

    res = bass_utils.run_bass_kernel_spmd(nc, in_maps, core_ids=list(range(8)))
    LAST_RESULT = res

    y = np.empty((B, S, D), dtype=np.float32)
    for b in range(B):
        acc = res.results[4 * b]["yT"].astype(np.float32)
        for g in range(1, 4):
            acc += res.results[4 * b + g]["yT"].astype(np.float32)
        y[b] = acc.T
    return y
